# revision 26
# baseline (speedup 1.0000x reference)
"""BatchPrototypeLoss on 8 TRN2 NeuronCores (Bass/Tile, data-parallel over N).

Math (equivalent to the reference):
  sums[c]  = segment_sum(x, t)            counts are irrelevant: cosine
  p_hat[c] = sums[c] / ||sums[c]||        similarity is scale-invariant
  sim[n,c] = (x[n]/||x[n]||) . p_hat[c]   bounded in [-1,1] -> softmax
                                          needs no max-subtraction
  loss = mean_n( log(sum_c exp(sim[n,c])) - sim[n, t_n] )
  sum_n sim[n,t_n] = sum_c <p_hat[c], segment_sum(x/||x||, t)[c]>
      -> the target-logit term needs no per-row gather; x_hat rides as 256
         extra rhs columns in the phase-1 one-hot segment matmul and is
         contracted against p_hat locally after the AllReduce.

Per core (8192 rows, fp16 data, fp32 accumulation):
  phase 1: stream x chunks; DVE sum(x^2) -> ACT sqrt -> DVE 1/x -> ACT
           x_hat; DVE builds one-hot(128x512) from an iota table; PE
           accumulates [sums | nsums] = onehot^T @ [x | x_hat] in PSUM.
  AllReduce raw sums only (512x256 fp16, warmed up by a tiny collective
           at kernel start); normalize -> p_hat; PE-transpose to [d,c].
  phase 2: dots = x^T-weights @ p_hatT per 128-row chunk; ACT exp with
           per-row 1/||x|| scale; DVE row-sum; final log + reductions.
  Output: [1,3] partial sums per core; host sums 24 values and divides
           by N (the gather/unshard step).
"""

import threading

import numpy as np

import concourse.bass as bass
import concourse.mybir as mybir
import concourse.tile as tile
from concourse import bacc
from concourse.bass_utils import run_bass_kernel_spmd
from concourse.masks import make_identity

N, D, C = 65536, 256, 512
NCORES = 8
R = N // NCORES      # 8192 rows per core
P = 128              # rows per chunk (partitions)
CH = R // P          # 64 chunks per core
CC = C // P          # 4 class chunks
DDim = D // P        # 2 feature chunks
GRP = 8              # chunks per sqrt/recip batch

BF16 = mybir.dt.bfloat16
F16 = mybir.dt.float16
F32 = mybir.dt.float32
ALU = mybir.AluOpType
AF = mybir.ActivationFunctionType

_cache = {}
_lock = threading.Lock()


def _build():
    nc = bacc.Bacc(
        "TRN2", target_bir_lowering=False, debug=False, num_devices=NCORES
    )
    x_d = nc.dram_tensor("x", [R, D], F16, kind="ExternalInput")
    xt_d = nc.dram_tensor("xt", [D, R], F16, kind="ExternalInput")
    tg_d = nc.dram_tensor("tg", [P, CH], F32, kind="ExternalInput")
    zout_d = nc.dram_tensor("zout", [1, 3], F32, kind="ExternalOutput")

    xsrc = x_d.ap().rearrange("(c p) d -> p c d", p=P)
    xtsrc = xt_d.ap().rearrange("(dd p) n -> p dd n", p=P)

    with tile.TileContext(nc) as tc:
        with (
            tc.tile_pool(name="const", bufs=1) as cpool,
            tc.tile_pool(name="work", bufs=1) as wpool,
            tc.tile_pool(name="dram", bufs=1, space="DRAM") as dpool,
            tc.tile_pool(name="oh", bufs=10) as ohpool,
            tc.tile_pool(name="scr", bufs=4) as spool,
            tc.tile_pool(name="epool", bufs=4) as epool,
            tc.tile_pool(name="psA", bufs=1, space="PSUM") as psA,
            tc.tile_pool(name="psB", bufs=1, space="PSUM") as psB,
            tc.tile_pool(name="psC", bufs=3, space="PSUM") as psC,
        ):
            # ---------------- constants / inputs resident in SBUF ---------
            iota = cpool.tile([P, C], F16, name="iota")
            nc.gpsimd.iota(
                iota[:],
                pattern=[[1, C]],
                base=0,
                channel_multiplier=0,
                allow_small_or_imprecise_dtypes=True,
            )
            ident = cpool.tile([P, P], F16, name="ident")
            make_identity(nc, ident[:])
            ones = cpool.tile([P, 1], F32, name="ones")
            nc.gpsimd.memset(ones[:], 1.0)

            tg_sb = cpool.tile([P, CH], F32, name="tg_sb")
            nc.sync.dma_start(tg_sb[:], tg_d.ap())

            # tiny warm-up collective: absorbs ncfw/channel init so the
            # real AllReduce later starts without the ~11us cold delay
            wu_in = dpool.tile([1, 16], F32, name="wu_in")
            wu_out = dpool.tile([1, 16], F32, name="wu_out", addr_space="Shared")
            wu_sb = cpool.tile([1, 16], F32, name="wu_sb")
            nc.gpsimd.memset(wu_sb[:], 0.0)
            nc.sync.dma_start(wu_in[:], wu_sb[:])
            nc.gpsimd.collective_compute(
                "AllReduce",
                ALU.add,
                replica_groups=[list(range(NCORES))],
                ins=[wu_in[:].opt()],
                outs=[wu_out[:].opt()],
            )

            # x resident as [P, chunk, [x | x_hat]] (fp16, 8 MB)
            xall = cpool.tile([P, CH, 2 * D], F16, name="xall")
            # x transposed resident as [P, dd, n] (fp16, 4 MB)
            xt_sb = cpool.tile([P, DDim, R], F16, name="xt_sb")

            SS = wpool.tile([P, CH], F32, name="SS")    # sum(x^2) per row
            SRT = wpool.tile([P, CH], F32, name="SRT")  # ||x|| per row
            INV = wpool.tile([P, CH], F32, name="INV")  # 1/||x|| per row

            # segment-sum accumulators: [class_chunk][128c, 512] f32
            # cols 0:256 = sums(x), cols 256:512 = sums(x_hat)
            seg = [
                psA.tile([P, 2 * D], F32, name=f"seg{cc}", tag=f"seg{cc}")
                for cc in range(CC)
            ]

            # ---------------- phase 1: local segment sums ------------------
            GBOUNDS = [0, 2, 6, 14, 22, 30, 38, 46, 56, 64]
            NG = len(GBOUNDS) - 1

            def emit_a(g):
                lo, hi = GBOUNDS[g], GBOUNDS[g + 1]
                for r in range(lo, hi):
                    nc.sync.dma_start(xall[:, r, 0:D], xsrc[:, r, :])
                    sq = spool.tile([P, D], F16, name="sqscr", tag="sqscr")
                    nc.vector.scalar_tensor_tensor(
                        out=sq[:],
                        in0=xall[:, r, 0:D],
                        scalar=1.0,
                        in1=xall[:, r, 0:D],
                        op0=ALU.mult,
                        op1=ALU.mult,
                        accum_out=SS[:, r : r + 1],
                    )
                nc.scalar.activation(SRT[:, lo:hi], SS[:, lo:hi], AF.Sqrt)
                nc.vector.reciprocal(INV[:, lo:hi], SRT[:, lo:hi])

            mm_marks = {}

            def emit_b(g):
                lo, hi = GBOUNDS[g], GBOUNDS[g + 1]
                for r in range(lo, hi):
                    nc.scalar.mul(
                        xall[:, r, D : 2 * D],
                        xall[:, r, 0:D],
                        INV[:, r : r + 1],
                    )
                    oh = ohpool.tile([P, C], F16, name="oh", tag="oh")
                    nc.vector.tensor_scalar(
                        out=oh[:],
                        in0=iota[:],
                        scalar1=tg_sb[:, r : r + 1],
                        scalar2=None,
                        op0=ALU.is_equal,
                    )
                    for cc in range(CC):
                        mi = nc.tensor.matmul(
                            seg[cc][:],
                            lhsT=oh[:, cc * P : (cc + 1) * P],
                            rhs=xall[:, r, :],
                            start=(r == 0),
                            stop=(r == CH - 1),
                        )
                    mm_marks[r] = mi

            emit_a(0)
            emit_a(1)
            for g in range(NG):
                emit_b(g)
                if g + 2 < NG:
                    emit_a(g + 2)

            from concourse.tile_rust import add_dep_helper as _adh

            NSPLIT = 8
            npc = R // NSPLIT
            for dd in range(DDim):
                for j in range(NSPLIT):
                    xti = nc.sync.dma_start(
                        xt_sb[:, dd, j * npc : (j + 1) * npc],
                        xtsrc[:, dd, j * npc : (j + 1) * npc],
                    )
                    mark = 28 + 4 * (dd * NSPLIT + j) // 2
                    _adh(
                        xti.ins,
                        mm_marks[mark].ins,
                        sync=True,
                        reason="stagger xt load into ph1 back half",
                    )


            # ---------------- AllReduce the raw segment sums --------------
            ar_in = dpool.tile([C, D], F16, name="ar_in")
            ar_out = dpool.tile([C, D], F16, name="ar_out", addr_space="Shared")
            sums_loc = wpool.tile([P, CC, D], F16, name="sums_loc")
            for cc in range(CC):
                nc.vector.tensor_copy(sums_loc[:, cc, :], seg[cc][:, 0:D])
            nc.scalar.dma_start(
                ar_in.rearrange("(cc p) d -> p cc d", p=P), sums_loc[:]
            )
            nc.gpsimd.collective_compute(
                "AllReduce",
                ALU.add,
                replica_groups=[list(range(NCORES))],
                ins=[ar_in[:].opt()],
                outs=[ar_out[:].opt()],
            )

            sums_sb = wpool.tile([P, CC, D], F16, name="sums_sb")
            nc.scalar.dma_start(
                sums_sb[:], ar_out.rearrange("(cc p) d -> p cc d", p=P)
            )

            # ---------------- prototypes: p_hat = sums/||sums|| -----------
            SSQ = wpool.tile([P, CC], F32, name="SSQ")
            NPR = wpool.tile([P, CC], F32, name="NPR")
            INPR = wpool.tile([P, CC], F32, name="INPR")
            for cc in range(CC):
                sq2 = spool.tile([P, D], F16, name="sqscr2", tag="sqscr")
                nc.vector.scalar_tensor_tensor(
                    out=sq2[:],
                    in0=sums_sb[:, cc, :],
                    scalar=1.0,
                    in1=sums_sb[:, cc, :],
                    op0=ALU.mult,
                    op1=ALU.mult,
                    accum_out=SSQ[:, cc : cc + 1],
                )
            nc.scalar.activation(NPR[:], SSQ[:], AF.Sqrt)
            nc.vector.reciprocal(INPR[:], NPR[:])

            phat = wpool.tile([P, CC, D], F16, name="phat")
            for cc in range(CC):
                nc.vector.tensor_scalar(
                    out=phat[:, cc, :],
                    in0=sums_sb[:, cc, :],
                    scalar1=INPR[:, cc : cc + 1],
                    scalar2=None,
                    op0=ALU.mult,
                )

            # local target-logit partial: -sum_c <p_hat_c, nsums_c>
            NEGS = wpool.tile([P, CC], F32, name="NEGS")
            for cc in range(CC):
                sq3 = spool.tile([P, D], F16, name="sqscr3", tag="sqscr")
                nc.vector.scalar_tensor_tensor(
                    out=sq3[:],
                    in0=seg[cc][:, D : 2 * D],
                    scalar=-1.0,
                    in1=phat[:, cc, :],
                    op0=ALU.mult,
                    op1=ALU.mult,
                    accum_out=NEGS[:, cc : cc + 1],
                )

            z = wpool.tile([P, 3], F32, name="z")
            nc.vector.reduce_sum(z[:, 2:3], NEGS[:], axis=mybir.AxisListType.X)

            # transpose p_hat [c,d] -> [d,c] for the dots matmul rhs
            phatT = wpool.tile([P, DDim, C], F16, name="phatT")
            for cc in range(CC):
                for dd in range(DDim):
                    tp = psB.tile([P, P], F16, name="tp", tag="tp")
                    nc.tensor.transpose(
                        tp[:], phat[:, cc, dd * P : (dd + 1) * P], ident[:]
                    )
                    nc.vector.tensor_copy(
                        phatT[:, dd, cc * P : (cc + 1) * P], tp[:]
                    )

            # ---------------- phase 2: dots + softmax denominator ---------
            S_sb = wpool.tile([P, CH], F32, name="S_sb")
            L_sb = wpool.tile([P, CH], F32, name="L_sb")
            for r in range(CH):
                dots = psC.tile([P, C], F32, name="dots", tag="dots")
                for dd in range(DDim):
                    nc.tensor.matmul(
                        dots[:],
                        lhsT=xt_sb[:, dd, r * P : (r + 1) * P],
                        rhs=phatT[:, dd, :],
                        start=(dd == 0),
                        stop=(dd == DDim - 1),
                    )
                e = epool.tile([P, C], F16, name="e", tag="e")
                nc.scalar.activation(
                    e[:], dots[:], AF.Exp, scale=INV[:, r : r + 1]
                )
                nc.vector.reduce_sum(
                    S_sb[:, r : r + 1], e[:], axis=mybir.AxisListType.X
                )
                if r == CH // 2:
                    nc.scalar.activation(
                        L_sb[:, 0 : CH // 2], S_sb[:, 0 : CH // 2], AF.Ln
                    )
                    nc.vector.reduce_sum(
                        z[:, 0:1],
                        L_sb[:, 0 : CH // 2],
                        axis=mybir.AxisListType.X,
                    )

            # ---------------- final reduction ------------------------------

            nc.scalar.activation(
                L_sb[:, CH // 2 :], S_sb[:, CH // 2 :], AF.Ln
            )
            nc.vector.reduce_sum(
                z[:, 1:2], L_sb[:, CH // 2 :], axis=mybir.AxisListType.X
            )
            zred = psC.tile([1, 3], F32, name="zred", tag="dots")
            nc.tensor.matmul(zred[:], lhsT=ones[:], rhs=z[:], start=True, stop=True)
            zsb = wpool.tile([1, 3], F32, name="zsb")
            nc.vector.tensor_copy(zsb[:], zred[:])
            nc.sync.dma_start(zout_d.ap(), zsb[:])

    nc.compile()
    return nc


def _get_nc():
    with _lock:
        if "nc" not in _cache:
            _cache["nc"] = _build()
        return _cache["nc"]


def _make_in_maps(inputs, targets):
    x = np.asarray(inputs, dtype=np.float32)
    t = np.asarray(targets, dtype=np.int32)
    in_maps = []
    for k in range(NCORES):
        sl = slice(k * R, (k + 1) * R)
        xs = x[sl]
        xb = xs.astype(np.float16)
        xtb = np.ascontiguousarray(xs.T).astype(np.float16)
        tgf = np.ascontiguousarray(
            t[sl].reshape(CH, P).T.astype(np.float32)
        )
        in_maps.append({"x": xb, "xt": xtb, "tg": tgf})
    return in_maps


def kernel(inputs, targets, _trace=False):
    nc = _get_nc()
    in_maps = _make_in_maps(inputs, targets)
    res = run_bass_kernel_spmd(
        nc, in_maps, core_ids=list(range(NCORES)), trace=_trace
    )
    if _trace:
        _cache["last_results"] = res
    ztot = np.sum([r["zout"] for r in res.results], dtype=np.float64)
    return np.asarray(ztot / N, dtype=np.float32)


# revision 27
# speedup vs baseline: 1.0234x; 1.0234x over previous
"""BatchPrototypeLoss on 8 TRN2 NeuronCores (Bass/Tile, data-parallel over N).

Math (equivalent to the reference):
  sums[c]  = segment_sum(x, t)            counts are irrelevant: cosine
  p_hat[c] = sums[c] / ||sums[c]||        similarity is scale-invariant
  sim[n,c] = (x[n]/||x[n]||) . p_hat[c]   bounded in [-1,1] -> softmax
                                          needs no max-subtraction
  loss = mean_n( log(sum_c exp(sim[n,c])) - sim[n, t_n] )
  sum_n sim[n,t_n] = sum_c <p_hat[c], segment_sum(x/||x||, t)[c]>
      -> the target-logit term needs no per-row gather; x_hat rides as 256
         extra rhs columns in the phase-1 one-hot segment matmul and is
         contracted against p_hat locally after the AllReduce.

Per core (8192 rows, fp16 data, fp32 accumulation):
  phase 1: stream x chunks; DVE sum(x^2) -> ACT sqrt -> DVE 1/x -> ACT
           x_hat; DVE builds one-hot(128x512) from an iota table; PE
           accumulates [sums | nsums] = onehot^T @ [x | x_hat] in PSUM.
  AllReduce raw sums only (512x256 fp16, warmed up by a tiny collective
           at kernel start); normalize -> p_hat; PE-transpose to [d,c].
  phase 2: dots = x^T-weights @ p_hatT per 128-row chunk; ACT exp with
           per-row 1/||x|| scale; DVE row-sum; final log + reductions.
  Output: [1,3] partial sums per core; host sums 24 values and divides
           by N (the gather/unshard step).
"""

import threading

import numpy as np

import concourse.bass as bass
import concourse.mybir as mybir
import concourse.tile as tile
from concourse import bacc
from concourse.bass_utils import run_bass_kernel_spmd
from concourse.masks import make_identity

N, D, C = 65536, 256, 512
NCORES = 8
R = N // NCORES      # 8192 rows per core
P = 128              # rows per chunk (partitions)
CH = R // P          # 64 chunks per core
CC = C // P          # 4 class chunks
DDim = D // P        # 2 feature chunks
GRP = 8              # chunks per sqrt/recip batch

BF16 = mybir.dt.bfloat16
F16 = mybir.dt.float16
F32 = mybir.dt.float32
ALU = mybir.AluOpType
AF = mybir.ActivationFunctionType

_cache = {}
_lock = threading.Lock()


def _build():
    nc = bacc.Bacc(
        "TRN2", target_bir_lowering=False, debug=False, num_devices=NCORES
    )
    x_d = nc.dram_tensor("x", [R, D], F16, kind="ExternalInput")
    xt_d = nc.dram_tensor("xt", [D, R], F16, kind="ExternalInput")
    tg_d = nc.dram_tensor("tg", [P, CH], F32, kind="ExternalInput")
    zout_d = nc.dram_tensor("zout", [1, 3], F32, kind="ExternalOutput")

    xsrc = x_d.ap().rearrange("(c p) d -> p c d", p=P)
    xtsrc = xt_d.ap().rearrange("(dd p) n -> p dd n", p=P)

    with tile.TileContext(nc) as tc:
        with (
            tc.tile_pool(name="const", bufs=1) as cpool,
            tc.tile_pool(name="work", bufs=1) as wpool,
            tc.tile_pool(name="dram", bufs=1, space="DRAM") as dpool,
            tc.tile_pool(name="oh", bufs=10) as ohpool,
            tc.tile_pool(name="scr", bufs=4) as spool,
            tc.tile_pool(name="epool", bufs=4) as epool,
            tc.tile_pool(name="psA", bufs=1, space="PSUM") as psA,
            tc.tile_pool(name="psB", bufs=1, space="PSUM") as psB,
            tc.tile_pool(name="psC", bufs=3, space="PSUM") as psC,
        ):
            # ---------------- constants / inputs resident in SBUF ---------
            iota = cpool.tile([P, C], F16, name="iota")
            nc.gpsimd.iota(
                iota[:],
                pattern=[[1, C]],
                base=0,
                channel_multiplier=0,
                allow_small_or_imprecise_dtypes=True,
            )
            ident = cpool.tile([P, P], F16, name="ident")
            make_identity(nc, ident[:])
            ones = cpool.tile([P, 1], F32, name="ones")
            nc.gpsimd.memset(ones[:], 1.0)

            tg_sb = cpool.tile([P, CH], F32, name="tg_sb")
            nc.sync.dma_start(tg_sb[:], tg_d.ap())

            # tiny warm-up collective: absorbs ncfw/channel init so the
            # real AllReduce later starts without the ~11us cold delay
            wu_in = dpool.tile([1, 16], F32, name="wu_in")
            wu_out = dpool.tile([1, 16], F32, name="wu_out", addr_space="Shared")
            wu_sb = cpool.tile([1, 16], F32, name="wu_sb")
            nc.gpsimd.memset(wu_sb[:], 0.0)
            nc.sync.dma_start(wu_in[:], wu_sb[:])
            nc.gpsimd.collective_compute(
                "AllReduce",
                ALU.add,
                replica_groups=[list(range(NCORES))],
                ins=[wu_in[:].opt()],
                outs=[wu_out[:].opt()],
            )

            # x resident as [P, chunk, [x | x_hat]] (fp16, 8 MB)
            xall = cpool.tile([P, CH, 2 * D], F16, name="xall")
            # x transposed resident as [P, dd, n] (fp16, 4 MB)
            xt_sb = cpool.tile([P, DDim, R], F16, name="xt_sb")

            SS = wpool.tile([P, CH], F32, name="SS")    # sum(x^2) per row
            SRT = wpool.tile([P, CH], F32, name="SRT")  # ||x|| per row
            INV = wpool.tile([P, CH], F32, name="INV")  # 1/||x|| per row

            # segment-sum accumulators: [class_chunk][128c, 512] f32
            # cols 0:256 = sums(x), cols 256:512 = sums(x_hat)
            seg = [
                psA.tile([P, 2 * D], F32, name=f"seg{cc}", tag=f"seg{cc}")
                for cc in range(CC)
            ]

            # ---------------- phase 1: local segment sums ------------------
            GBOUNDS = [0, 2, 6, 14, 22, 30, 38, 46, 56, 64]
            NG = len(GBOUNDS) - 1

            def emit_a(g):
                lo, hi = GBOUNDS[g], GBOUNDS[g + 1]
                for r in range(lo, hi):
                    nc.sync.dma_start(xall[:, r, 0:D], xsrc[:, r, :])
                    sq = spool.tile([P, D], F16, name="sqscr", tag="sqscr")
                    nc.vector.scalar_tensor_tensor(
                        out=sq[:],
                        in0=xall[:, r, 0:D],
                        scalar=1.0,
                        in1=xall[:, r, 0:D],
                        op0=ALU.mult,
                        op1=ALU.mult,
                        accum_out=SS[:, r : r + 1],
                    )
                nc.scalar.activation(SRT[:, lo:hi], SS[:, lo:hi], AF.Sqrt)
                nc.vector.reciprocal(INV[:, lo:hi], SRT[:, lo:hi])

            mm_marks = {}

            def emit_b(g):
                lo, hi = GBOUNDS[g], GBOUNDS[g + 1]
                for r in range(lo, hi):
                    nc.scalar.mul(
                        xall[:, r, D : 2 * D],
                        xall[:, r, 0:D],
                        INV[:, r : r + 1],
                    )
                    oh = ohpool.tile([P, C], F16, name="oh", tag="oh")
                    nc.vector.tensor_scalar(
                        out=oh[:],
                        in0=iota[:],
                        scalar1=tg_sb[:, r : r + 1],
                        scalar2=None,
                        op0=ALU.is_equal,
                    )
                    for cc in range(CC):
                        mi = nc.tensor.matmul(
                            seg[cc][:],
                            lhsT=oh[:, cc * P : (cc + 1) * P],
                            rhs=xall[:, r, :],
                            start=(r == 0),
                            stop=(r == CH - 1),
                        )
                    mm_marks[r] = mi

            emit_a(0)
            emit_a(1)
            for g in range(NG):
                emit_b(g)
                if g + 2 < NG:
                    emit_a(g + 2)

            from concourse.tile_rust import add_dep_helper as _adh

            NSPLIT = 8
            npc = R // NSPLIT
            for dd in range(DDim):
                for j in range(NSPLIT):
                    xti = nc.sync.dma_start(
                        xt_sb[:, dd, j * npc : (j + 1) * npc],
                        xtsrc[:, dd, j * npc : (j + 1) * npc],
                    )
                    mark = 28 + 4 * (dd * NSPLIT + j) // 2
                    _adh(
                        xti.ins,
                        mm_marks[mark].ins,
                        sync=True,
                        reason="stagger xt load into ph1 back half",
                    )


            # ---------------- AllReduce the raw segment sums --------------
            ar_in = dpool.tile([C, D], F16, name="ar_in")
            ar_out = dpool.tile([C, D], F16, name="ar_out", addr_space="Shared")
            sums_loc = wpool.tile([P, CC, D], F16, name="sums_loc")
            for cc in range(CC):
                nc.scalar.copy(sums_loc[:, cc, :], seg[cc][:, 0:D])
            nc.scalar.dma_start(
                ar_in.rearrange("(cc p) d -> p cc d", p=P), sums_loc[:]
            )
            nc.gpsimd.collective_compute(
                "AllReduce",
                ALU.add,
                replica_groups=[list(range(NCORES))],
                ins=[ar_in[:].opt()],
                outs=[ar_out[:].opt()],
            )

            sums_sb = wpool.tile([P, CC, D], F16, name="sums_sb")
            nc.scalar.dma_start(
                sums_sb[:], ar_out.rearrange("(cc p) d -> p cc d", p=P)
            )

            # ---------------- prototypes: p_hat = sums/||sums|| -----------
            SSQ = wpool.tile([P, CC], F32, name="SSQ")
            NPR = wpool.tile([P, CC], F32, name="NPR")
            INPR = wpool.tile([P, CC], F32, name="INPR")
            for cc in range(CC):
                sq2 = spool.tile([P, D], F16, name="sqscr2", tag="sqscr")
                nc.vector.scalar_tensor_tensor(
                    out=sq2[:],
                    in0=sums_sb[:, cc, :],
                    scalar=1.0,
                    in1=sums_sb[:, cc, :],
                    op0=ALU.mult,
                    op1=ALU.mult,
                    accum_out=SSQ[:, cc : cc + 1],
                )
            nc.scalar.activation(NPR[:], SSQ[:], AF.Sqrt)
            nc.vector.reciprocal(INPR[:], NPR[:])

            phat = wpool.tile([P, CC, D], F16, name="phat")
            for cc in range(CC):
                nc.vector.tensor_scalar(
                    out=phat[:, cc, :],
                    in0=sums_sb[:, cc, :],
                    scalar1=INPR[:, cc : cc + 1],
                    scalar2=None,
                    op0=ALU.mult,
                )

            NEGS = wpool.tile([P, CC], F32, name="NEGS")
            z = wpool.tile([P, 3], F32, name="z")

            # transpose p_hat [c,d] -> [d,c] for the dots matmul rhs
            phatT = wpool.tile([P, DDim, C], F16, name="phatT")
            for cc in range(CC):
                for dd in range(DDim):
                    tp = psB.tile([P, P], F16, name="tp", tag="tp")
                    nc.tensor.transpose(
                        tp[:], phat[:, cc, dd * P : (dd + 1) * P], ident[:]
                    )
                    nc.vector.tensor_copy(
                        phatT[:, dd, cc * P : (cc + 1) * P], tp[:]
                    )

            # ---------------- phase 2: dots + softmax denominator ---------
            S_sb = wpool.tile([P, CH], F32, name="S_sb")
            L_sb = wpool.tile([P, CH], F32, name="L_sb")
            for r in range(CH):
                dots = psC.tile([P, C], F32, name="dots", tag="dots")
                for dd in range(DDim):
                    nc.tensor.matmul(
                        dots[:],
                        lhsT=xt_sb[:, dd, r * P : (r + 1) * P],
                        rhs=phatT[:, dd, :],
                        start=(dd == 0),
                        stop=(dd == DDim - 1),
                    )
                e = epool.tile([P, C], F16, name="e", tag="e")
                nc.scalar.activation(
                    e[:], dots[:], AF.Exp, scale=INV[:, r : r + 1]
                )
                nc.vector.reduce_sum(
                    S_sb[:, r : r + 1], e[:], axis=mybir.AxisListType.X
                )
                if r == 2:
                    # local target-logit partial: -sum_c <p_hat_c, nsums_c>
                    for cc in range(CC):
                        sq3 = spool.tile(
                            [P, D], F16, name="sqscr3", tag="sqscr"
                        )
                        nc.vector.scalar_tensor_tensor(
                            out=sq3[:],
                            in0=seg[cc][:, D : 2 * D],
                            scalar=-1.0,
                            in1=phat[:, cc, :],
                            op0=ALU.mult,
                            op1=ALU.mult,
                            accum_out=NEGS[:, cc : cc + 1],
                        )
                    nc.vector.reduce_sum(
                        z[:, 2:3], NEGS[:], axis=mybir.AxisListType.X
                    )
                if r == CH // 2:
                    nc.scalar.activation(
                        L_sb[:, 0 : CH // 2], S_sb[:, 0 : CH // 2], AF.Ln
                    )
                    nc.vector.reduce_sum(
                        z[:, 0:1],
                        L_sb[:, 0 : CH // 2],
                        axis=mybir.AxisListType.X,
                    )

            # ---------------- final reduction ------------------------------

            nc.scalar.activation(
                L_sb[:, CH // 2 :], S_sb[:, CH // 2 :], AF.Ln
            )
            nc.vector.reduce_sum(
                z[:, 1:2], L_sb[:, CH // 2 :], axis=mybir.AxisListType.X
            )
            zred = psC.tile([1, 3], F32, name="zred", tag="dots")
            nc.tensor.matmul(zred[:], lhsT=ones[:], rhs=z[:], start=True, stop=True)
            zsb = wpool.tile([1, 3], F32, name="zsb")
            nc.vector.tensor_copy(zsb[:], zred[:])
            nc.sync.dma_start(zout_d.ap(), zsb[:])

    nc.compile()
    return nc


def _get_nc():
    with _lock:
        if "nc" not in _cache:
            _cache["nc"] = _build()
        return _cache["nc"]


def _make_in_maps(inputs, targets):
    x = np.asarray(inputs, dtype=np.float32)
    t = np.asarray(targets, dtype=np.int32)
    in_maps = []
    for k in range(NCORES):
        sl = slice(k * R, (k + 1) * R)
        xs = x[sl]
        xb = xs.astype(np.float16)
        xtb = np.ascontiguousarray(xs.T).astype(np.float16)
        tgf = np.ascontiguousarray(
            t[sl].reshape(CH, P).T.astype(np.float32)
        )
        in_maps.append({"x": xb, "xt": xtb, "tg": tgf})
    return in_maps


def kernel(inputs, targets, _trace=False):
    nc = _get_nc()
    in_maps = _make_in_maps(inputs, targets)
    res = run_bass_kernel_spmd(
        nc, in_maps, core_ids=list(range(NCORES)), trace=_trace
    )
    if _trace:
        _cache["last_results"] = res
    ztot = np.sum([r["zout"] for r in res.results], dtype=np.float64)
    return np.asarray(ztot / N, dtype=np.float32)


# revision 28
# speedup vs baseline: 1.1488x; 1.1225x over previous
"""BatchPrototypeLoss on 8 TRN2 NeuronCores (Bass/Tile, data-parallel over N).

Math (equivalent to the reference):
  sums[c]  = segment_sum(x, t)            counts are irrelevant: cosine
  p_hat[c] = sums[c] / ||sums[c]||        similarity is scale-invariant
  sim[n,c] = (x[n]/||x[n]||) . p_hat[c]   bounded in [-1,1] -> softmax
                                          needs no max-subtraction
  loss = mean_n( log(sum_c exp(sim[n,c])) - sim[n, t_n] )
  sum_n sim[n,t_n] = sum_c <p_hat[c], segment_sum(x/||x||, t)[c]>
      -> the target-logit term needs no per-row gather; x_hat rides as 256
         extra rhs columns in the phase-1 one-hot segment matmul and is
         contracted against p_hat locally after the AllReduce.

Per core (8192 rows, fp16 data, fp32 accumulation):
  phase 1: stream x chunks; DVE sum(x^2) -> ACT sqrt -> DVE 1/x -> ACT
           x_hat; DVE builds one-hot(128x512) from an iota table; PE
           accumulates [sums | nsums] = onehot^T @ [x | x_hat] in PSUM.
  AllReduce raw sums only (512x256 fp16, warmed up by a tiny collective
           at kernel start); normalize -> p_hat; PE-transpose to [d,c].
  phase 2: dots = x^T-weights @ p_hatT per 128-row chunk; ACT exp with
           per-row 1/||x|| scale; DVE row-sum; final log + reductions.
  Output: [1,3] partial sums per core; host sums 24 values and divides
           by N (the gather/unshard step).
"""

import threading

import numpy as np

import concourse.bass as bass
import concourse.mybir as mybir
import concourse.tile as tile
from concourse import bacc
from concourse.bass_utils import run_bass_kernel_spmd
from concourse.masks import make_identity

N, D, C = 65536, 256, 512
NCORES = 8
R = N // NCORES      # 8192 rows per core
P = 128              # rows per chunk (partitions)
CH = R // P          # 64 chunks per core
CC = C // P          # 4 class chunks
DDim = D // P        # 2 feature chunks
GRP = 8              # chunks per sqrt/recip batch

BF16 = mybir.dt.bfloat16
F16 = mybir.dt.float16
F32 = mybir.dt.float32
ALU = mybir.AluOpType
AF = mybir.ActivationFunctionType

_cache = {}
_lock = threading.Lock()


def _build():
    nc = bacc.Bacc(
        "TRN2", target_bir_lowering=False, debug=False, num_devices=NCORES
    )
    x_d = nc.dram_tensor("x", [R, D], F16, kind="ExternalInput")
    xt_d = nc.dram_tensor("xt", [D, R], F16, kind="ExternalInput")
    tg_d = nc.dram_tensor("tg", [P, CH], F32, kind="ExternalInput")
    zout_d = nc.dram_tensor("zout", [1, 3], F32, kind="ExternalOutput")

    xsrc = x_d.ap().rearrange("(c p) d -> p c d", p=P)
    xtsrc = xt_d.ap().rearrange("(dd p) n -> p dd n", p=P)

    with tile.TileContext(nc) as tc:
        with (
            tc.tile_pool(name="const", bufs=1) as cpool,
            tc.tile_pool(name="work", bufs=1) as wpool,
            tc.tile_pool(name="dram", bufs=1, space="DRAM") as dpool,
            tc.tile_pool(name="oh", bufs=10) as ohpool,
            tc.tile_pool(name="scr", bufs=4) as spool,
            tc.tile_pool(name="epool", bufs=4) as epool,
            tc.tile_pool(name="psA", bufs=1, space="PSUM") as psA,
            tc.tile_pool(name="psB", bufs=1, space="PSUM") as psB,
            tc.tile_pool(name="psC", bufs=3, space="PSUM") as psC,
        ):
            # ---------------- constants / inputs resident in SBUF ---------
            iota = cpool.tile([P, C], F16, name="iota")
            nc.gpsimd.iota(
                iota[:],
                pattern=[[1, C]],
                base=0,
                channel_multiplier=0,
                allow_small_or_imprecise_dtypes=True,
            )
            ident = cpool.tile([P, P], F16, name="ident")
            make_identity(nc, ident[:])
            ones = cpool.tile([P, 1], F32, name="ones")
            nc.gpsimd.memset(ones[:], 1.0)

            tg_sb = cpool.tile([P, CH], F32, name="tg_sb")
            nc.sync.dma_start(tg_sb[:], tg_d.ap())

            # tiny warm-up collective: absorbs ncfw/channel init so the
            # real AllReduce later starts without the ~11us cold delay
            wu_in = dpool.tile([1, 16], F32, name="wu_in")
            wu_out = dpool.tile([1, 16], F32, name="wu_out", addr_space="Shared")
            wu_sb = cpool.tile([1, 16], F32, name="wu_sb")
            nc.gpsimd.memset(wu_sb[:], 0.0)
            nc.sync.dma_start(wu_in[:], wu_sb[:])
            nc.gpsimd.collective_compute(
                "AllReduce",
                ALU.add,
                replica_groups=[list(range(NCORES))],
                ins=[wu_in[:].opt()],
                outs=[wu_out[:].opt()],
            )

            # x resident as [P, chunk, [x | x_hat]] (fp16, 8 MB)
            xall = cpool.tile([P, CH, 2 * D], F16, name="xall")
            # x transposed resident as [P, dd, n] (fp16, 4 MB)
            xt_sb = cpool.tile([P, DDim, R], F16, name="xt_sb")

            SS = wpool.tile([P, CH], F32, name="SS")    # sum(x^2) per row
            SRT = wpool.tile([P, CH], F32, name="SRT")  # ||x|| per row
            INV = wpool.tile([P, CH], F32, name="INV")  # 1/||x|| per row

            # segment-sum accumulators: [class_chunk][128c, 512] f32
            # cols 0:256 = sums(x), cols 256:512 = sums(x_hat)
            seg = [
                psA.tile([P, 2 * D], F32, name=f"seg{cc}", tag=f"seg{cc}")
                for cc in range(CC)
            ]

            # ---------------- phase 1: local segment sums ------------------
            GBOUNDS = [0, 2, 6, 14, 22, 30, 38, 46, 56, 64]
            NG = len(GBOUNDS) - 1

            def emit_a(g):
                lo, hi = GBOUNDS[g], GBOUNDS[g + 1]
                for r in range(lo, hi):
                    nc.sync.dma_start(xall[:, r, 0:D], xsrc[:, r, :])
                    sq = spool.tile([P, D], F16, name="sqscr", tag="sqscr")
                    nc.vector.scalar_tensor_tensor(
                        out=sq[:],
                        in0=xall[:, r, 0:D],
                        scalar=1.0,
                        in1=xall[:, r, 0:D],
                        op0=ALU.mult,
                        op1=ALU.mult,
                        accum_out=SS[:, r : r + 1],
                    )
                nc.scalar.activation(SRT[:, lo:hi], SS[:, lo:hi], AF.Sqrt)
                nc.vector.reciprocal(INV[:, lo:hi], SRT[:, lo:hi])

            mm_marks = {}

            def emit_b(g):
                lo, hi = GBOUNDS[g], GBOUNDS[g + 1]
                for r in range(lo, hi):
                    nc.scalar.mul(
                        xall[:, r, D : 2 * D],
                        xall[:, r, 0:D],
                        INV[:, r : r + 1],
                    )
                    oh = ohpool.tile([P, C], F16, name="oh", tag="oh")
                    nc.vector.tensor_scalar(
                        out=oh[:],
                        in0=iota[:],
                        scalar1=tg_sb[:, r : r + 1],
                        scalar2=None,
                        op0=ALU.is_equal,
                    )
                    for cc in range(CC):
                        mi = nc.tensor.matmul(
                            seg[cc][:],
                            lhsT=oh[:, cc * P : (cc + 1) * P],
                            rhs=xall[:, r, :],
                            start=(r == 0),
                            stop=(r == CH - 1),
                        )
                    mm_marks[r] = mi

            emit_a(0)
            emit_a(1)
            for g in range(NG):
                emit_b(g)
                if g + 2 < NG:
                    emit_a(g + 2)

            from concourse.tile_rust import add_dep_helper as _adh

            NSPLIT = 8
            npc = R // NSPLIT
            for dd in range(DDim):
                for j in range(NSPLIT):
                    xti = nc.sync.dma_start(
                        xt_sb[:, dd, j * npc : (j + 1) * npc],
                        xtsrc[:, dd, j * npc : (j + 1) * npc],
                    )
                    mark = 28 + 4 * (dd * NSPLIT + j) // 2
                    _adh(
                        xti.ins,
                        mm_marks[mark].ins,
                        sync=True,
                        reason="stagger xt load into ph1 back half",
                    )


            # ---------------- AllReduce the raw segment sums --------------
            ar_in = dpool.tile([C, D], F16, name="ar_in")
            ar_out = dpool.tile([C, D], F16, name="ar_out", addr_space="Shared")
            sums_loc = wpool.tile([P, CC, D], F16, name="sums_loc")
            for cc in range(CC):
                nc.vector.tensor_copy(sums_loc[:, cc, :], seg[cc][:, 0:D])
            nc.scalar.dma_start(
                ar_in.rearrange("(cc p) d -> p cc d", p=P), sums_loc[:]
            )
            nc.gpsimd.collective_compute(
                "AllReduce",
                ALU.add,
                replica_groups=[list(range(NCORES))],
                ins=[ar_in[:].opt()],
                outs=[ar_out[:].opt()],
            )

            sums_sb = wpool.tile([P, CC, D], F16, name="sums_sb")
            nc.scalar.dma_start(
                sums_sb[:], ar_out.rearrange("(cc p) d -> p cc d", p=P)
            )

            # ---------------- prototypes: p_hat = sums/||sums|| -----------
            SSQ = wpool.tile([P, CC], F32, name="SSQ")
            NPR = wpool.tile([P, CC], F32, name="NPR")
            INPR = wpool.tile([P, CC], F32, name="INPR")
            for cc in range(CC):
                sq2 = spool.tile([P, D], F16, name="sqscr2", tag="sqscr")
                nc.vector.scalar_tensor_tensor(
                    out=sq2[:],
                    in0=sums_sb[:, cc, :],
                    scalar=1.0,
                    in1=sums_sb[:, cc, :],
                    op0=ALU.mult,
                    op1=ALU.mult,
                    accum_out=SSQ[:, cc : cc + 1],
                )
            nc.scalar.activation(NPR[:], SSQ[:], AF.Sqrt)
            nc.vector.reciprocal(INPR[:], NPR[:])

            phat = wpool.tile([P, CC, D], F16, name="phat")
            for cc in range(CC):
                nc.vector.tensor_scalar(
                    out=phat[:, cc, :],
                    in0=sums_sb[:, cc, :],
                    scalar1=INPR[:, cc : cc + 1],
                    scalar2=None,
                    op0=ALU.mult,
                )

            # local target-logit partial: -sum_c <p_hat_c, nsums_c>
            NEGS = wpool.tile([P, CC], F32, name="NEGS")
            for cc in range(CC):
                sq3 = spool.tile([P, D], F16, name="sqscr3", tag="sqscr")
                nc.vector.scalar_tensor_tensor(
                    out=sq3[:],
                    in0=seg[cc][:, D : 2 * D],
                    scalar=-1.0,
                    in1=phat[:, cc, :],
                    op0=ALU.mult,
                    op1=ALU.mult,
                    accum_out=NEGS[:, cc : cc + 1],
                )

            z = wpool.tile([P, 3], F32, name="z")
            nc.vector.reduce_sum(z[:, 2:3], NEGS[:], axis=mybir.AxisListType.X)

            # transpose p_hat [c,d] -> [d,c] for the dots matmul rhs
            phatT = wpool.tile([P, DDim, C], F16, name="phatT")
            for cc in range(CC):
                for dd in range(DDim):
                    tp = psB.tile([P, P], F16, name="tp", tag="tp")
                    nc.tensor.transpose(
                        tp[:], phat[:, cc, dd * P : (dd + 1) * P], ident[:]
                    )
                    nc.vector.tensor_copy(
                        phatT[:, dd, cc * P : (cc + 1) * P], tp[:]
                    )

            # ---------------- phase 2: dots + softmax denominator ---------
            S_sb = wpool.tile([P, CH], F32, name="S_sb")
            L_sb = wpool.tile([P, CH], F32, name="L_sb")
            for r in range(CH):
                dots = psC.tile([P, C], F32, name="dots", tag="dots")
                for dd in range(DDim):
                    nc.tensor.matmul(
                        dots[:],
                        lhsT=xt_sb[:, dd, r * P : (r + 1) * P],
                        rhs=phatT[:, dd, :],
                        start=(dd == 0),
                        stop=(dd == DDim - 1),
                    )
                e = epool.tile([P, C], F16, name="e", tag="e")
                nc.scalar.activation(
                    e[:], dots[:], AF.Exp, scale=INV[:, r : r + 1]
                )
                nc.vector.reduce_sum(
                    S_sb[:, r : r + 1], e[:], axis=mybir.AxisListType.X
                )
                if r == CH // 2:
                    nc.scalar.activation(
                        L_sb[:, 0 : CH // 2], S_sb[:, 0 : CH // 2], AF.Ln
                    )
                    nc.vector.reduce_sum(
                        z[:, 0:1],
                        L_sb[:, 0 : CH // 2],
                        axis=mybir.AxisListType.X,
                    )

            # ---------------- final reduction ------------------------------

            nc.scalar.activation(
                L_sb[:, CH // 2 :], S_sb[:, CH // 2 :], AF.Ln
            )
            nc.vector.reduce_sum(
                z[:, 1:2], L_sb[:, CH // 2 :], axis=mybir.AxisListType.X
            )
            zred = psC.tile([1, 3], F32, name="zred", tag="dots")
            nc.tensor.matmul(zred[:], lhsT=ones[:], rhs=z[:], start=True, stop=True)
            zsb = wpool.tile([1, 3], F32, name="zsb")
            nc.vector.tensor_copy(zsb[:], zred[:])
            nc.sync.dma_start(zout_d.ap(), zsb[:])

    nc.compile()
    return nc


def _get_nc():
    with _lock:
        if "nc" not in _cache:
            _cache["nc"] = _build()
        return _cache["nc"]


def _make_in_maps(inputs, targets):
    x = np.asarray(inputs, dtype=np.float32)
    t = np.asarray(targets, dtype=np.int32)
    in_maps = []
    for k in range(NCORES):
        sl = slice(k * R, (k + 1) * R)
        xs = x[sl]
        xb = xs.astype(np.float16)
        xtb = np.ascontiguousarray(xs.T).astype(np.float16)
        tgf = np.ascontiguousarray(
            t[sl].reshape(CH, P).T.astype(np.float32)
        )
        in_maps.append({"x": xb, "xt": xtb, "tg": tgf})
    return in_maps


def kernel(inputs, targets, _trace=False):
    nc = _get_nc()
    in_maps = _make_in_maps(inputs, targets)
    res = run_bass_kernel_spmd(
        nc, in_maps, core_ids=list(range(NCORES)), trace=_trace
    )
    if _trace:
        _cache["last_results"] = res
    ztot = np.sum([r["zout"] for r in res.results], dtype=np.float64)
    return np.asarray(ztot / N, dtype=np.float32)


# revision 29
# speedup vs baseline: 1.1657x; 1.0147x over previous
"""BatchPrototypeLoss on 8 TRN2 NeuronCores (Bass/Tile, data-parallel over N).

Math (equivalent to the reference):
  sums[c]  = segment_sum(x, t)            counts are irrelevant: cosine
  p_hat[c] = sums[c] / ||sums[c]||        similarity is scale-invariant
  sim[n,c] = (x[n]/||x[n]||) . p_hat[c]   bounded in [-1,1] -> softmax
                                          needs no max-subtraction
  loss = mean_n( log(sum_c exp(sim[n,c])) - sim[n, t_n] )
  sum_n sim[n,t_n] = sum_c <p_hat[c], segment_sum(x/||x||, t)[c]>
      -> the target-logit term needs no per-row gather; x_hat rides as 256
         extra rhs columns in the phase-1 one-hot segment matmul and is
         contracted against p_hat locally after the AllReduce.

Per core (8192 rows, fp16 data, fp32 accumulation):
  phase 1: stream x chunks; DVE sum(x^2) -> ACT sqrt -> DVE 1/x -> ACT
           x_hat; DVE builds one-hot(128x512) from an iota table; PE
           accumulates [sums | nsums] = onehot^T @ [x | x_hat] in PSUM.
  AllReduce raw sums only (512x256 fp16, warmed up by a tiny collective
           at kernel start); normalize -> p_hat; PE-transpose to [d,c].
  phase 2: dots = x^T-weights @ p_hatT per 128-row chunk; ACT exp with
           per-row 1/||x|| scale; DVE row-sum; final log + reductions.
  Output: [1,3] partial sums per core; host sums 24 values and divides
           by N (the gather/unshard step).
"""

import threading

import numpy as np

import concourse.bass as bass
import concourse.mybir as mybir
import concourse.tile as tile
from concourse import bacc
from concourse.bass_utils import run_bass_kernel_spmd
from concourse.masks import make_identity

N, D, C = 65536, 256, 512
NCORES = 8
R = N // NCORES      # 8192 rows per core
P = 128              # rows per chunk (partitions)
CH = R // P          # 64 chunks per core
CC = C // P          # 4 class chunks
DDim = D // P        # 2 feature chunks
GRP = 8              # chunks per sqrt/recip batch

BF16 = mybir.dt.bfloat16
F16 = mybir.dt.float16
F32 = mybir.dt.float32
ALU = mybir.AluOpType
AF = mybir.ActivationFunctionType

_cache = {}
_lock = threading.Lock()


def _build():
    nc = bacc.Bacc(
        "TRN2", target_bir_lowering=False, debug=False, num_devices=NCORES
    )
    x_d = nc.dram_tensor("x", [R, D], F16, kind="ExternalInput")
    xt_d = nc.dram_tensor("xt", [D, R], F16, kind="ExternalInput")
    tg_d = nc.dram_tensor("tg", [P, CH], F32, kind="ExternalInput")
    zout_d = nc.dram_tensor("zout", [1, 3], F32, kind="ExternalOutput")

    xsrc = x_d.ap().rearrange("(c p) d -> p c d", p=P)
    xtsrc = xt_d.ap().rearrange("(dd p) n -> p dd n", p=P)

    with tile.TileContext(nc) as tc:
        with (
            tc.tile_pool(name="const", bufs=1) as cpool,
            tc.tile_pool(name="work", bufs=1) as wpool,
            tc.tile_pool(name="dram", bufs=1, space="DRAM") as dpool,
            tc.tile_pool(name="oh", bufs=10) as ohpool,
            tc.tile_pool(name="scr", bufs=4) as spool,
            tc.tile_pool(name="epool", bufs=4) as epool,
            tc.tile_pool(name="psA", bufs=1, space="PSUM") as psA,
            tc.tile_pool(name="psB", bufs=1, space="PSUM") as psB,
            tc.tile_pool(name="psC", bufs=3, space="PSUM") as psC,
        ):
            # ---------------- constants / inputs resident in SBUF ---------
            iota = cpool.tile([P, C], F16, name="iota")
            nc.gpsimd.iota(
                iota[:],
                pattern=[[1, C]],
                base=0,
                channel_multiplier=0,
                allow_small_or_imprecise_dtypes=True,
            )
            ident = cpool.tile([P, P], F16, name="ident")
            make_identity(nc, ident[:])
            ones = cpool.tile([P, 1], F32, name="ones")
            nc.gpsimd.memset(ones[:], 1.0)

            tg_sb = cpool.tile([P, CH], F32, name="tg_sb")
            nc.sync.dma_start(tg_sb[:], tg_d.ap())

            # tiny warm-up collective: absorbs ncfw/channel init so the
            # real AllReduce later starts without the ~11us cold delay
            wu_in = dpool.tile([1, 16], F32, name="wu_in")
            wu_out = dpool.tile([1, 16], F32, name="wu_out", addr_space="Shared")
            wu_sb = cpool.tile([1, 16], F32, name="wu_sb")
            nc.gpsimd.memset(wu_sb[:], 0.0)
            nc.sync.dma_start(wu_in[:], wu_sb[:])
            nc.gpsimd.collective_compute(
                "AllReduce",
                ALU.add,
                replica_groups=[list(range(NCORES))],
                ins=[wu_in[:].opt()],
                outs=[wu_out[:].opt()],
            )

            # x resident as [P, chunk, [x | x_hat]] (fp16, 8 MB)
            xall = cpool.tile([P, CH, 2 * D], F16, name="xall")
            # x transposed resident as [P, dd, n] (fp16, 4 MB)
            xt_sb = cpool.tile([P, DDim, R], F16, name="xt_sb")

            SS = wpool.tile([P, CH], F32, name="SS")    # sum(x^2) per row
            SRT = wpool.tile([P, CH], F32, name="SRT")  # ||x|| per row
            INV = wpool.tile([P, CH], F32, name="INV")  # 1/||x|| per row

            # segment-sum accumulators: [class_chunk][128c, 512] f32
            # cols 0:256 = sums(x), cols 256:512 = sums(x_hat)
            seg = [
                psA.tile([P, 2 * D], F32, name=f"seg{cc}", tag=f"seg{cc}")
                for cc in range(CC)
            ]

            # ---------------- phase 1: local segment sums ------------------
            GBOUNDS = [0, 2, 6, 14, 22, 30, 38, 46, 56, 64]
            NG = len(GBOUNDS) - 1

            def emit_a(g):
                lo, hi = GBOUNDS[g], GBOUNDS[g + 1]
                for r in range(lo, hi, 2):
                    nc.sync.dma_start(
                        xall[:, r : r + 2, 0:D], xsrc[:, r : r + 2, :]
                    )
                for r in range(lo, hi):
                    sq = spool.tile([P, D], F16, name="sqscr", tag="sqscr")
                    nc.vector.scalar_tensor_tensor(
                        out=sq[:],
                        in0=xall[:, r, 0:D],
                        scalar=1.0,
                        in1=xall[:, r, 0:D],
                        op0=ALU.mult,
                        op1=ALU.mult,
                        accum_out=SS[:, r : r + 1],
                    )
                nc.scalar.activation(SRT[:, lo:hi], SS[:, lo:hi], AF.Sqrt)
                nc.vector.reciprocal(INV[:, lo:hi], SRT[:, lo:hi])

            mm_marks = {}

            def emit_b(g):
                lo, hi = GBOUNDS[g], GBOUNDS[g + 1]
                for r in range(lo, hi):
                    nc.scalar.mul(
                        xall[:, r, D : 2 * D],
                        xall[:, r, 0:D],
                        INV[:, r : r + 1],
                    )
                    oh = ohpool.tile([P, C], F16, name="oh", tag="oh")
                    nc.vector.tensor_scalar(
                        out=oh[:],
                        in0=iota[:],
                        scalar1=tg_sb[:, r : r + 1],
                        scalar2=None,
                        op0=ALU.is_equal,
                    )
                    for cc in range(CC):
                        mi = nc.tensor.matmul(
                            seg[cc][:],
                            lhsT=oh[:, cc * P : (cc + 1) * P],
                            rhs=xall[:, r, :],
                            start=(r == 0),
                            stop=(r == CH - 1),
                        )
                    mm_marks[r] = mi

            emit_a(0)
            emit_a(1)
            for g in range(NG):
                emit_b(g)
                if g + 2 < NG:
                    emit_a(g + 2)

            from concourse.tile_rust import add_dep_helper as _adh

            NSPLIT = 8
            npc = R // NSPLIT
            for dd in range(DDim):
                for j in range(NSPLIT):
                    xti = nc.sync.dma_start(
                        xt_sb[:, dd, j * npc : (j + 1) * npc],
                        xtsrc[:, dd, j * npc : (j + 1) * npc],
                    )
                    mark = 28 + 4 * (dd * NSPLIT + j) // 2
                    _adh(
                        xti.ins,
                        mm_marks[mark].ins,
                        sync=True,
                        reason="stagger xt load into ph1 back half",
                    )


            # ---------------- AllReduce the raw segment sums --------------
            ar_in = dpool.tile([C, D], F16, name="ar_in")
            ar_out = dpool.tile([C, D], F16, name="ar_out", addr_space="Shared")
            sums_loc = wpool.tile([P, CC, D], F16, name="sums_loc")
            for cc in range(CC):
                nc.vector.tensor_copy(sums_loc[:, cc, :], seg[cc][:, 0:D])
            nc.scalar.dma_start(
                ar_in.rearrange("(cc p) d -> p cc d", p=P), sums_loc[:]
            )
            nc.gpsimd.collective_compute(
                "AllReduce",
                ALU.add,
                replica_groups=[list(range(NCORES))],
                ins=[ar_in[:].opt()],
                outs=[ar_out[:].opt()],
            )

            sums_sb = wpool.tile([P, CC, D], F16, name="sums_sb")
            nc.scalar.dma_start(
                sums_sb[:], ar_out.rearrange("(cc p) d -> p cc d", p=P)
            )

            # ---------------- prototypes: p_hat = sums/||sums|| -----------
            SSQ = wpool.tile([P, CC], F32, name="SSQ")
            NPR = wpool.tile([P, CC], F32, name="NPR")
            INPR = wpool.tile([P, CC], F32, name="INPR")
            for cc in range(CC):
                sq2 = spool.tile([P, D], F16, name="sqscr2", tag="sqscr")
                nc.vector.scalar_tensor_tensor(
                    out=sq2[:],
                    in0=sums_sb[:, cc, :],
                    scalar=1.0,
                    in1=sums_sb[:, cc, :],
                    op0=ALU.mult,
                    op1=ALU.mult,
                    accum_out=SSQ[:, cc : cc + 1],
                )
            nc.scalar.activation(NPR[:], SSQ[:], AF.Sqrt)
            nc.vector.reciprocal(INPR[:], NPR[:])

            phat = wpool.tile([P, CC, D], F16, name="phat")
            for cc in range(CC):
                nc.vector.tensor_scalar(
                    out=phat[:, cc, :],
                    in0=sums_sb[:, cc, :],
                    scalar1=INPR[:, cc : cc + 1],
                    scalar2=None,
                    op0=ALU.mult,
                )

            # local target-logit partial: -sum_c <p_hat_c, nsums_c>
            NEGS = wpool.tile([P, CC], F32, name="NEGS")
            for cc in range(CC):
                sq3 = spool.tile([P, D], F16, name="sqscr3", tag="sqscr")
                nc.vector.scalar_tensor_tensor(
                    out=sq3[:],
                    in0=seg[cc][:, D : 2 * D],
                    scalar=-1.0,
                    in1=phat[:, cc, :],
                    op0=ALU.mult,
                    op1=ALU.mult,
                    accum_out=NEGS[:, cc : cc + 1],
                )

            z = wpool.tile([P, 3], F32, name="z")
            nc.vector.reduce_sum(z[:, 2:3], NEGS[:], axis=mybir.AxisListType.X)

            # transpose p_hat [c,d] -> [d,c] for the dots matmul rhs
            phatT = wpool.tile([P, DDim, C], F16, name="phatT")
            for cc in range(CC):
                for dd in range(DDim):
                    tp = psB.tile([P, P], F16, name="tp", tag="tp")
                    nc.tensor.transpose(
                        tp[:], phat[:, cc, dd * P : (dd + 1) * P], ident[:]
                    )
                    nc.vector.tensor_copy(
                        phatT[:, dd, cc * P : (cc + 1) * P], tp[:]
                    )

            # ---------------- phase 2: dots + softmax denominator ---------
            S_sb = wpool.tile([P, CH], F32, name="S_sb")
            L_sb = wpool.tile([P, CH], F32, name="L_sb")
            for r in range(CH):
                dots = psC.tile([P, C], F32, name="dots", tag="dots")
                for dd in range(DDim):
                    nc.tensor.matmul(
                        dots[:],
                        lhsT=xt_sb[:, dd, r * P : (r + 1) * P],
                        rhs=phatT[:, dd, :],
                        start=(dd == 0),
                        stop=(dd == DDim - 1),
                    )
                e = epool.tile([P, C], F16, name="e", tag="e")
                nc.scalar.activation(
                    e[:], dots[:], AF.Exp, scale=INV[:, r : r + 1]
                )
                nc.vector.reduce_sum(
                    S_sb[:, r : r + 1], e[:], axis=mybir.AxisListType.X
                )
                if r == CH // 2:
                    nc.scalar.activation(
                        L_sb[:, 0 : CH // 2], S_sb[:, 0 : CH // 2], AF.Ln
                    )
                    nc.vector.reduce_sum(
                        z[:, 0:1],
                        L_sb[:, 0 : CH // 2],
                        axis=mybir.AxisListType.X,
                    )

            # ---------------- final reduction ------------------------------

            nc.scalar.activation(
                L_sb[:, CH // 2 :], S_sb[:, CH // 2 :], AF.Ln
            )
            nc.vector.reduce_sum(
                z[:, 1:2], L_sb[:, CH // 2 :], axis=mybir.AxisListType.X
            )
            zred = psC.tile([1, 3], F32, name="zred", tag="dots")
            nc.tensor.matmul(zred[:], lhsT=ones[:], rhs=z[:], start=True, stop=True)
            zsb = wpool.tile([1, 3], F32, name="zsb")
            nc.vector.tensor_copy(zsb[:], zred[:])
            nc.sync.dma_start(zout_d.ap(), zsb[:])

    nc.compile()
    return nc


def _get_nc():
    with _lock:
        if "nc" not in _cache:
            _cache["nc"] = _build()
        return _cache["nc"]


def _make_in_maps(inputs, targets):
    x = np.asarray(inputs, dtype=np.float32)
    t = np.asarray(targets, dtype=np.int32)
    in_maps = []
    for k in range(NCORES):
        sl = slice(k * R, (k + 1) * R)
        xs = x[sl]
        xb = xs.astype(np.float16)
        xtb = np.ascontiguousarray(xs.T).astype(np.float16)
        tgf = np.ascontiguousarray(
            t[sl].reshape(CH, P).T.astype(np.float32)
        )
        in_maps.append({"x": xb, "xt": xtb, "tg": tgf})
    return in_maps


def kernel(inputs, targets, _trace=False):
    nc = _get_nc()
    in_maps = _make_in_maps(inputs, targets)
    res = run_bass_kernel_spmd(
        nc, in_maps, core_ids=list(range(NCORES)), trace=_trace
    )
    if _trace:
        _cache["last_results"] = res
    ztot = np.sum([r["zout"] for r in res.results], dtype=np.float64)
    return np.asarray(ztot / N, dtype=np.float32)
